# revision 1
# baseline (speedup 1.0000x reference)
# nn_DifferenceCost kernel for Trainium2 (Bass), 8-core SPMD.  v3
#
# out[b,s,y,x] = ||ref[b,:,y,x] - tgt[b,:,y+oy,x+ox]||_2, 0 out of bounds.
# s_val = nr2 + nt2 - 2*cross via TensorEngine: per 128-pixel block
# (16 rows x 8 cols), mm1 computes cross against the 24x16 target halo
# window (384 streamed fp16 columns, K=128); mm2 (K=2) accumulates
# -(nt2[n] + nr2[m])/2 so PSUM = -s_val/2.  Four x-adjacent blocks share
# one 4-bank PSUM tile; a single ACT pass computes Sqrt(-2*PSUM) for all
# four into the fp16 staging slab (out-of-bounds entries produce finite
# garbage that the host masks to zero by geometry).  Inputs double-buffer
# and prefetch one rep ahead, trickled in per slab so band dumps keep DMA
# priority; SP-issued DMAs dump per-row-pair band windows (160 of 384
# window cols x 4 blocks, contiguous 1280B runs) while compute continues;
# the host performs the band->output gather (pure data movement).
#
# Sync-slot workaround: the target ISA encodes one semaphore wait per
# instruction, but Tile emits several on some (PSUM-WAR + RAW; the
# kernel-tail drain collects every DMA lane).  _legalize_waits hoists
# excess waits onto inserted same-engine NoOps, preserving the exact
# synchronization one wait at a time.
import sys

if "/opt/trn_rl_repo" not in sys.path:
    sys.path.insert(0, "/opt/trn_rl_repo")

import numpy as np

import concourse.bass as bass
import concourse.mybir as mybir
from concourse import tile

F16 = mybir.dt.float16
F32 = mybir.dt.float32

B, C, H, W = 4, 128, 96, 192
D = 4                    # max displacement
NS = 9                   # shifts per axis
S = NS * NS              # 81
NY = 48                  # output rows per core
GY, GX = NY + 2 * D, W + 2 * D   # 56 x 200 target halo grid
BRY, BRX = 16, 8         # ref block: 16 rows x 8 cols = 128 pixels
WRY, WRX = BRY + 2 * D, BRX + 2 * D  # 24 x 16 target window
NW = WRY * WRX           # 384 streamed columns per block
NSLAB = NY // BRY        # 3 slabs of 16 rows
NXB = W // BRX           # 24 x-blocks
NBLK = NSLAB * NXB       # 72 blocks per core
NPAIR = BRY // 2         # 8 row-pairs per block
PBW = 10 * WRX           # 160: band window per row-pair (rows 2pg..2pg+10)
SENTINEL = -60000.0      # fp16-safe OOB filler; keeps sqrt input positive


# engine schedule for the sqrt stage: one entry per 4-block unit within a
# slab (6 units).  Sqrt only exists on the Activation engine (DVE/Pool
# have no pow ALU op, and Pool cannot read PSUM), so all units go to "A";
# the knob remains for experiments.
SQRT_SCHED = ["A"] * 6


def build_program(shrink: bool = True, reps: int = 1,
                  sched: list[str] | None = None) -> bass.Bass:
    sched = sched or SQRT_SCHED
    nc = bass.Bass()
    tgt_d = nc.declare_dram_parameter("tgt", [C, GY, GX], F16, isOutput=False)
    ref_d = nc.declare_dram_parameter("ref", [C, NBLK, 128], F16, isOutput=False)
    tm2_d = nc.declare_dram_parameter("tm2", [2, GY, GX], F16, isOutput=False)
    rr2_d = nc.declare_dram_parameter("rr2", [2, NBLK, 128], F16, isOutput=False)
    NU = NXB // 4          # 6 four-block units per slab
    out_d = nc.declare_dram_parameter(
        "out", [NSLAB, NPAIR, 16, NU, 4 * PBW], F16, isOutput=True)

    # osb layout: element (p, s0, u, n, h) at p*OSB_F + s0*SLAB_F + u*1536
    # + n*4 + h -- each unit's sqrt dst is one contiguous 1536-elem run (no
    # overlap between units, keeps Tile deps parallel), and each row-pair
    # band (160 window cols x 4 blocks) is a contiguous 1280B descriptor.
    OSB_F = NBLK * NW
    SLAB_F = NW * NXB      # 9216 elems per slab per partition
    PW = 512               # f32 elems per PSUM bank (bank-aligned quarters)

    with tile.TileContext(nc) as tc:
        with (
            tc.tile_pool(name="big", bufs=1) as big,
            tc.tile_pool(name="inp", bufs=2) as inp,
            tc.tile_pool(name="pa", bufs=2, space="PSUM") as pap,
        ):
            ROW_CHUNKS = [(0, WRY), (WRY, BRY), (WRY + BRY, BRY)]

            def alloc_in():
                return (inp.tile([C, GY, GX], F16, name="tgt_sb"),
                        inp.tile([C, NBLK, 128], F16, name="ref_sb"),
                        inp.tile([2, GY, GX], F16, name="tm2_sb"),
                        inp.tile([2, NBLK, 128], F16, name="rr2_sb"))

            def emit_load_chunk(t, s0):
                """Issue the input DMAs feeding slab s0 (plus the small norm
                tensors with slab 0).  Chunked so prefetch for the next rep
                trickles in behind the current rep's band dumps instead of
                monopolizing the DMA engines at rep start."""
                tgt_sb, ref_sb, tm2_sb, rr2_sb = t
                if s0 == 0:
                    nc.sync.dma_start(tm2_sb[:], tm2_d[:])
                    nc.sync.dma_start(rr2_sb[:], rr2_d[:])
                lo, n = ROW_CHUNKS[s0]
                nc.sync.dma_start(tgt_sb[:, lo:lo + n, :],
                                  tgt_d[:, lo:lo + n, :])
                bl, bh = s0 * NXB, (s0 + 1) * NXB
                nc.sync.dma_start(ref_sb[:, bl:bh, :], ref_d[:, bl:bh, :])

            cur = alloc_in()
            for s0 in range(NSLAB):
                emit_load_chunk(cur, s0)
            for rep in range(reps):
              tgt_sb, ref_sb, tm2_sb, rr2_sb = cur
              if rep + 1 < reps:
                  cur = alloc_in()   # prefetch next rep into the other bufs

              for s0 in range(NSLAB):
                if rep + 1 < reps:
                    emit_load_chunk(cur, s0)
                oslab = big.tile([C, SLAB_F], F16, name="oslab", bufs=2)
                for u in range(NU):
                    # four x-adjacent blocks share one 4-bank PSUM tile
                    pa = pap.tile([128, 4 * PW], F32)
                    for h in range(4):
                        xb = 4 * u + h
                        blk = s0 * NXB + xb
                        tgt_sl = tgt_sb[:, s0 * BRY:s0 * BRY + WRY,
                                        xb * BRX:xb * BRX + WRX]
                        tm2_sl = tm2_sb[:, s0 * BRY:s0 * BRY + WRY,
                                        xb * BRX:xb * BRX + WRX]
                        po = pa[:, h * PW:h * PW + NW]
                        # mm2 first: it takes the PSUM-slot WAR wait (its
                        # operands are long-resident), so mm1 carries only
                        # the tgt-chunk RAW -- <=1 wait per instruction.
                        nc.tensor.matmul(po, rr2_sb[:, blk, :], tm2_sl,
                                         start=True, stop=False)
                        nc.tensor.matmul(po, ref_sb[:, blk, :], tgt_sl,
                                         start=False, stop=True)
                    # one fused sqrt(-2*PSUM) over the unit's four blocks
                    # (1536 elems, strided across the four PSUM banks).  OOB
                    # entries produce finite garbage (sentinel keeps the
                    # sqrt argument positive), discarded by the host's band
                    # gather + geometric mask.
                    src = bass.AP(pa.tensor, pa.offset,
                                  [[4 * PW, 128], [PW, 4], [1, NW]])
                    dst = bass.AP(oslab.tensor, u * 4 * NW,
                                  [[SLAB_F, 128], [1, 4], [4, NW]])
                    eng = sched[u % len(sched)]
                    if eng == "A":
                        nc.scalar.activation(
                            dst, src,
                            mybir.ActivationFunctionType.Sqrt, scale=-2.0)
                    else:
                        e = nc.vector if eng == "D" else nc.gpsimd
                        e.tensor_scalar(dst, src, -2.0, 0.5,
                                        mybir.AluOpType.mult,
                                        mybir.AluOpType.pow)
                # banded dumps via SP HWDGE: row-pair pg only ever needs
                # window cols [32pg, 32pg+160) of each unit -- one 640B
                # contiguous run per (partition, unit).
                for pg in range(NPAIR):
                    src = bass.AP(
                        oslab.tensor,
                        16 * pg * SLAB_F + 128 * pg,
                        [[SLAB_F, 16], [4 * NW, NU], [1, 4 * PBW]],
                    )
                    nc.sync.dma_start(out=out_d[s0, pg], in_=src)

    if shrink:
        _legalize_waits(nc)
    return nc


def _legalize_waits(nc) -> None:
    """The target ISA encodes at most ONE semaphore wait per instruction,
    but Tile emits instructions with several (PSUM-slot WAR + data RAW on
    hot ops; the kernel-tail drain collects every lane).  Hoist all but
    one wait of each such instruction onto freshly inserted same-engine
    NoOps placed immediately before it: the engine executes the NoOps'
    waits in program order, so the synchronization is preserved exactly,
    one wait per instruction."""
    for f in nc.m.functions:
        for b in f.blocks:
            il = b.instructions
            idx = 0
            while idx < len(il):
                ins = il[idx]
                si = ins.sync_info
                nw = len(si.on_wait) if si and si.on_wait else 0
                if nw > 1:
                    waits = list(si.on_wait)
                    for w in waits[:-1]:
                        nop = nc.engines[ins.engine].nop()
                        nop_ins = nop.ins if hasattr(nop, "ins") else nop
                        removed = False
                        for bb2 in f.blocks:
                            lst = bb2.instructions
                            if lst and lst[-1].name == nop_ins.name:
                                lst.pop()
                                removed = True
                                break
                        assert removed, "could not relocate wait NoOp"
                        nop_ins.sync_info = mybir.SyncInfo(
                            on_wait=[w], on_update=[])
                        il.insert(idx, nop_ins)
                        idx += 1
                    ins.sync_info = mybir.SyncInfo(
                        on_wait=[waits[-1]], on_update=si.on_update)
                idx += 1


def make_in_maps(reference_fm: np.ndarray, target_fm: np.ndarray):
    rh = reference_fm.astype(np.float16)
    th = target_fm.astype(np.float16)
    nr2 = (rh.astype(np.float32) ** 2).sum(axis=1)  # [B, H, W]
    nt2 = (th.astype(np.float32) ** 2).sum(axis=1)
    in_maps = []
    for c in range(8):
        b, half = c // 2, c % 2
        y0 = half * NY
        r_lo, r_hi = max(0, y0 - D), min(H, y0 + NY + D)
        g_lo = r_lo - (y0 - D)

        tgt_slab = np.zeros((C, GY, GX), np.float16)
        tgt_slab[:, g_lo:g_lo + (r_hi - r_lo), D:D + W] = th[b, :, r_lo:r_hi, :]

        tm2 = np.zeros((2, GY, GX), np.float32)
        tm2[0] = SENTINEL
        tm2[0, g_lo:g_lo + (r_hi - r_lo), D:D + W] = -0.5 * nt2[b, r_lo:r_hi, :]
        tm2[1] = 1.0

        # block-major ref: [C, blk, p] with blk = s0*24+xb, p = ry*8+rx
        ref_slab = rh[b, :, y0:y0 + NY, :].reshape(C, NSLAB, BRY, NXB, BRX)
        ref_slab = np.ascontiguousarray(
            ref_slab.transpose(0, 1, 3, 2, 4).reshape(C, NBLK, 128))

        nr_core = nr2[b, y0:y0 + NY, :]                    # [48, 192]
        rblk = nr_core.reshape(NSLAB, BRY, NXB, BRX)       # [s0, ry, xb, rx]
        rblk = rblk.transpose(0, 2, 1, 3).reshape(NBLK, 128)
        rr2 = np.stack([np.ones((NBLK, 128), np.float32), -0.5 * rblk])

        in_maps.append({
            "tgt": tgt_slab,
            "ref": ref_slab,
            "tm2": tm2.astype(np.float16),
            "rr2": rr2.astype(np.float16),
        })
    return in_maps


# ---- host-side band gather (pure data movement) ----
# out value for shift (soy, sox) at block pixel (ry, rx):
#   pair pg = ry//2, partition-in-pair pp = (ry%2)*8+rx,
#   band col = (ry+soy)*16 + (rx+sox) - 32*pg  (in [0, 160))
_RYg = np.arange(BRY)[None, :, None, None]
_RXg = np.arange(BRX)[None, None, None, :]
_SOYg = np.arange(NS)[:, None, None, None]
_SOXg = np.arange(NS)[None, None, :, None]
_PG = np.broadcast_to(_RYg // 2, (NS, BRY, NS, BRX))
_PP = np.broadcast_to((_RYg % 2) * 8 + _RXg, (NS, BRY, NS, BRX))
_COL = (_RYg + _SOYg) * WRX + (_RXg + _SOXg) - 32 * (_RYg // 2)


def assemble(results) -> np.ndarray:
    out = np.zeros((B, S, H, W), np.float32)
    for c in range(8):
        b, half = c // 2, c % 2
        o = np.asarray(results[c]["out"]).astype(np.float32)
        # device layout is [.., unit, band-col, half]; fold (unit, half)
        # back into xb and put band-col last as the gather indices expect
        o = (o.reshape(NSLAB, NPAIR, 16, NXB // 4, PBW, 4)
              .transpose(0, 1, 2, 3, 5, 4)
              .reshape(NSLAB, NPAIR, 16, NXB, PBW))
        # g[soy, ry, sox, rx, s0, xb] = o[s0, pg, pp, xb, col]
        g = o[:, _PG, _PP, :, _COL]
        # fancy-index result: [9,16,9,8, NSLAB, NXB]
        g = g.transpose(4, 0, 2, 1, 5, 3)        # [s0,soy,sox,ry,xb,rx]
        g = g.transpose(1, 2, 0, 3, 4, 5).reshape(S, NY, W)
        out[b, :, half * NY:half * NY + NY, :] = g
    # zero the out-of-bounds border of each shift (geometry only)
    for soy in range(NS):
        for sox in range(NS):
            s = soy * NS + sox
            oy, ox = soy - D, sox - D
            if oy < 0:
                out[:, s, :-oy, :] = 0.0
            elif oy > 0:
                out[:, s, H - oy:, :] = 0.0
            if ox < 0:
                out[:, s, :, :-ox] = 0.0
            elif ox > 0:
                out[:, s, :, W - ox:] = 0.0
    return out


_PROGRAM = None


def kernel(reference_fm: np.ndarray, target_fm: np.ndarray) -> np.ndarray:
    global _PROGRAM
    from concourse.bass_utils import run_bass_kernel_spmd

    reference_fm = np.asarray(reference_fm, dtype=np.float32)
    target_fm = np.asarray(target_fm, dtype=np.float32)
    if _PROGRAM is None:
        _PROGRAM = build_program()
    in_maps = make_in_maps(reference_fm, target_fm)
    res = run_bass_kernel_spmd(_PROGRAM, in_maps, core_ids=list(range(8)))
    return assemble(res.results)



# revision 4
# speedup vs baseline: 9.9281x; 9.9281x over previous
# nn_DifferenceCost kernel for Trainium2 (Bass), 8-core SPMD.  v4
#
# out[b,s,y,x] = ||ref[b,:,y,x] - tgt[b,:,y+oy,x+ox]||_2, 0 out of bounds.
#
# The device computes ONLY the cross term: per 128-pixel block (16 rows x
# 8 cols), one TensorEngine matmul against the 24x16 target halo window
# (384 streamed fp16 columns, K=128) gives PSUM[pixel, window] = cross.
# PSUM is copied to an fp16 staging slab (Activation/DVE engines
# alternating), and per-row-pair band windows (160 of 384 window cols x 4
# blocks, contiguous 1280B runs) are dumped by SP-issued DMAs.
#
# Everything else happens on the host in two jax-CPU jitted functions
# (compiled once, multithreaded, fused):
#   _prep:   f32->f16 casts, zero-padding, per-core slabs, |r|^2 / |t|^2
#            channel norms (from the SAME quantized values the device
#            sees, so the identity ||r-t||^2 = nr2 + nt2 - 2 cross is
#            exact up to fp16 input rounding).
#   _finish: flat-index gather band -> [s,y,x], s = nr2 + nt2 - 2 cross,
#            sqrt, geometric border mask.
#
# The compiled device executable is cached in a module global and reused
# across kernel() calls (inputs stream in per call; output buffers are
# donated and chained call-to-call).
#
# Sync-slot workaround: the target ISA encodes one semaphore wait per
# instruction, but Tile emits several on some (PSUM-WAR + RAW; the
# kernel-tail drain collects every DMA lane).  _legalize_waits hoists
# excess waits onto inserted same-engine NoOps, preserving the exact
# synchronization one wait at a time.
import sys

if "/opt/trn_rl_repo" not in sys.path:
    sys.path.insert(0, "/opt/trn_rl_repo")

import numpy as np

import concourse.bass as bass
import concourse.mybir as mybir
from concourse import tile

F16 = mybir.dt.float16
F32 = mybir.dt.float32

B, C, H, W = 4, 128, 96, 192
D = 4                    # max displacement
NS = 9                   # shifts per axis
S = NS * NS              # 81
NY = 48                  # output rows per core
GY, GX = NY + 2 * D, W + 2 * D   # 56 x 200 target halo grid
BRY, BRX = 16, 8         # ref block: 16 rows x 8 cols = 128 pixels
WRY, WRX = BRY + 2 * D, BRX + 2 * D  # 24 x 16 target window
NW = WRY * WRX           # 384 streamed columns per block
NSLAB = NY // BRY        # 3 slabs of 16 rows
NXB = W // BRX           # 24 x-blocks
NBLK = NSLAB * NXB       # 72 blocks per core
NPAIR = BRY // 2         # 8 row-pairs per block
PBW = 10 * WRX           # 160: band window per row-pair (rows 2pg..2pg+10)
NU = NXB // 4            # 6 four-block units per slab
SLAB_F = NW * NXB        # 9216 staging elems per slab per partition
PW = 512                 # f32 elems per PSUM bank (bank-aligned quarters)
BAND_N = NPAIR * 16 * NU * 4 * PBW   # elems per slab in the band dump

# engine schedule for the PSUM->SBUF copy, one entry per 4-block unit
# ("A" = Activation, "D" = DVE); both engines can read PSUM, Pool cannot.
COPY_SCHED = ["A", "D"]


def build_program(reps: int = 1, shrink: bool = True) -> bass.Bass:
    nc = bass.Bass()
    tgt_d = nc.declare_dram_parameter("tgt", [C, GY, GX], F16, isOutput=False)
    ref_d = nc.declare_dram_parameter("ref", [C, NBLK, 128], F16,
                                      isOutput=False)
    out_d = nc.declare_dram_parameter(
        "out", [NSLAB, NPAIR, 16, NU, 4 * PBW], F16, isOutput=True)

    with tile.TileContext(nc) as tc:
        with (
            tc.tile_pool(name="big", bufs=1) as big,
            tc.tile_pool(name="inp", bufs=2) as inp,
            tc.tile_pool(name="pa", bufs=2, space="PSUM") as pap,
        ):
            # tgt rows used by slab s0: [16 s0, 16 s0 + 24)
            TGT_CHUNKS = [(0, WRY), (WRY, BRY), (WRY + BRY, BRY)]

            def alloc_in():
                return (inp.tile([C, GY, GX], F16, name="tgt_sb"),
                        inp.tile([C, NBLK, 128], F16, name="ref_sb"))

            def emit_load_chunk(t, s0):
                """Input DMAs feeding slab s0, chunked so next-rep prefetch
                trickles in behind the current rep's band dumps."""
                tgt_sb, ref_sb = t
                lo, n = TGT_CHUNKS[s0]
                nc.sync.dma_start(tgt_sb[:, lo:lo + n, :],
                                  tgt_d[:, lo:lo + n, :])
                bl, bh = s0 * NXB, (s0 + 1) * NXB
                nc.sync.dma_start(ref_sb[:, bl:bh, :], ref_d[:, bl:bh, :])

            cur = alloc_in()
            for s0 in range(NSLAB):
                emit_load_chunk(cur, s0)
            for rep in range(reps):
              tgt_sb, ref_sb = cur
              if rep + 1 < reps:
                  cur = alloc_in()   # prefetch next rep into the other bufs
              for s0 in range(NSLAB):
                if rep + 1 < reps:
                    emit_load_chunk(cur, s0)
                oslab = big.tile([C, SLAB_F], F16, name="oslab", bufs=2)
                for u in range(NU):
                    # four x-adjacent blocks share one 4-bank PSUM tile
                    pa = pap.tile([128, 4 * PW], F32)
                    for h in range(4):
                        xb = 4 * u + h
                        tgt_sl = tgt_sb[:, s0 * BRY:s0 * BRY + WRY,
                                        xb * BRX:xb * BRX + WRX]
                        po = pa[:, h * PW:h * PW + NW]
                        nc.tensor.matmul(po, ref_sb[:, s0 * NXB + xb, :],
                                         tgt_sl, start=True, stop=True)
                    # copy the unit's four blocks PSUM -> fp16 staging slab
                    # (1536 elems, h interleaved at stride 1 so each
                    # row-pair band is one contiguous 1280B dump run).
                    src = bass.AP(pa.tensor, pa.offset,
                                  [[4 * PW, 128], [PW, 4], [1, NW]])
                    dst = bass.AP(oslab.tensor, u * 4 * NW,
                                  [[SLAB_F, 128], [1, 4], [4, NW]])
                    eng = COPY_SCHED[(s0 * NU + u) % len(COPY_SCHED)]
                    if eng == "A":
                        nc.scalar.activation(
                            dst, src, mybir.ActivationFunctionType.Copy)
                    else:
                        nc.vector.tensor_scalar_add(dst, src, 0.0)
                # banded dumps via SP HWDGE: row-pair pg only ever needs
                # window cols [32pg, 32pg+160) of each unit -- one 1280B
                # contiguous run per (partition, unit).
                for pg in range(NPAIR):
                    src = bass.AP(
                        oslab.tensor,
                        16 * pg * SLAB_F + 128 * pg,
                        [[SLAB_F, 16], [4 * NW, NU], [1, 4 * PBW]],
                    )
                    nc.sync.dma_start(out=out_d[s0, pg], in_=src)

    if shrink:
        _legalize_waits(nc)
    return nc


def _legalize_waits(nc) -> None:
    """The target ISA encodes at most ONE semaphore wait per instruction,
    but Tile emits instructions with several (PSUM-slot WAR + data RAW on
    hot ops; the kernel-tail drain collects every lane).  Hoist all but
    one wait of each such instruction onto freshly inserted same-engine
    NoOps placed immediately before it: the engine executes the NoOps'
    waits in program order, so the synchronization is preserved exactly,
    one wait per instruction."""
    for f in nc.m.functions:
        for b in f.blocks:
            il = b.instructions
            idx = 0
            while idx < len(il):
                ins = il[idx]
                si = ins.sync_info
                nw = len(si.on_wait) if si and si.on_wait else 0
                if nw > 1:
                    waits = list(si.on_wait)
                    for w in waits[:-1]:
                        nop = nc.engines[ins.engine].nop()
                        nop_ins = nop.ins if hasattr(nop, "ins") else nop
                        removed = False
                        for bb2 in f.blocks:
                            lst = bb2.instructions
                            if lst and lst[-1].name == nop_ins.name:
                                lst.pop()
                                removed = True
                                break
                        assert removed, "could not relocate wait NoOp"
                        nop_ins.sync_info = mybir.SyncInfo(
                            on_wait=[w], on_update=[])
                        il.insert(idx, nop_ins)
                        idx += 1
                    ins.sync_info = mybir.SyncInfo(
                        on_wait=[waits[-1]], on_update=si.on_update)
                idx += 1


# ---- host side: gather indices + geometric mask (built once at import) ----
def _build_idx() -> np.ndarray:
    """IDX[s, y, x] = flat index into a core's band dump [NSLAB*BAND_N]
    holding cross(pixel (y,x), shift s)."""
    soy = np.arange(NS)[:, None, None, None]
    sox = np.arange(NS)[None, :, None, None]
    y = np.arange(NY)[None, None, :, None]
    x = np.arange(W)[None, None, None, :]
    s0, ry = y // BRY, y % BRY
    rx = x % BRX
    pg = ry // 2
    pp = (ry % 2) * 8 + rx
    xb = x // BRX
    u, hh = xb // 4, xb % 4
    n = (ry + soy) * WRX + (rx + sox)
    j = 4 * (n - 32 * pg) + hh
    idx = s0 * BAND_N + ((pg * 16 + pp) * NU + u) * (4 * PBW) + j
    return np.broadcast_to(idx, (NS, NS, NY, W)).reshape(S, NY, W) \
             .astype(np.int32)


def _build_mask() -> np.ndarray:
    m = np.zeros((NS, NS, H, W), np.bool_)
    for soy in range(NS):
        for sox in range(NS):
            oy, ox = soy - D, sox - D
            m[soy, sox,
              max(0, -oy):H - max(0, oy),
              max(0, -ox):W - max(0, ox)] = True
    return m.reshape(S, H, W)


_IDX = _build_idx()
_MASK = _build_mask()

_PREP = None
_FINISH = None
_RUN = None


def _get_host_fns():
    global _PREP, _FINISH
    if _PREP is not None:
        return _PREP, _FINISH
    import jax
    import jax.numpy as jnp

    def prep(ref, tgt):
        rh = ref.astype(jnp.float16)
        th = tgt.astype(jnp.float16)
        nr2 = jnp.sum(jnp.square(rh.astype(jnp.float32)), axis=1)
        nt2 = jnp.sum(jnp.square(th.astype(jnp.float32)), axis=1)
        # per-core ref slabs, core = 2b + half, block-major [C, NBLK,
        # 128] (the matmul weights AP must be 2D: one free dimension)
        r = (rh.reshape(B, C, 2, NSLAB, BRY, NXB, BRX)
               .transpose(0, 2, 1, 3, 5, 4, 6)
               .reshape(8 * C, NBLK, 128))
        # per-core zero-padded target halo slabs [C, GY, GX]
        tp = jnp.pad(th, ((0, 0), (0, 0), (D, D), (D, D)))
        t = jnp.stack([tp[:, :, 0:GY, :], tp[:, :, NY:NY + GY, :]], axis=1)
        t = t.reshape(8 * C, GY, GX)
        nt2p = jnp.pad(nt2, ((0, 0), (D, D), (D, D)))
        return r, t, nr2, nt2p

    def finish(bands, nr2, nt2p):
        # bands: [8 cores * NSLAB, NPAIR, 16, NU, 4*PBW] f16
        flat = bands.reshape(8, NSLAB * BAND_N)
        g = jnp.take(flat, _IDX.reshape(-1), axis=1)
        g = g.reshape(8, S, NY, W).astype(jnp.float32)
        g = (g.reshape(B, 2, S, NY, W).transpose(0, 2, 1, 3, 4)
              .reshape(B, S, H, W))
        wins = jnp.stack([nt2p[:, a:a + H, b:b + W]
                          for a in range(NS) for b in range(NS)], axis=1)
        sv = nr2[:, None] + wins - 2.0 * g
        return jnp.where(_MASK & (sv > 0),
                         jnp.sqrt(jnp.maximum(sv, 1e-30)), 0.0)

    _PREP = jax.jit(prep, backend="cpu")
    _FINISH = jax.jit(finish, backend="cpu")
    return _PREP, _FINISH


def _get_runner():
    """Compile the device program once into a cached jitted callable.
    Output buffers are donated and chained call-to-call; inputs stream in
    per call."""
    global _RUN
    if _RUN is not None:
        return _RUN
    import jax
    from jax.sharding import Mesh, PartitionSpec
    from jax.experimental.shard_map import shard_map
    import concourse.bass2jax as b2j

    b2j.install_neuronx_cc_hook()
    nc = build_program(reps=1)
    n_cores = 8
    partition_name = (nc.partition_id_tensor.name
                      if nc.partition_id_tensor else None)
    in_names, out_names, out_avals, zero_outs = [], [], [], []
    for alloc in nc.m.functions[0].allocations:
        if not isinstance(alloc, mybir.MemoryLocationSet):
            continue
        name = alloc.memorylocations[0].name
        if alloc.kind == "ExternalInput":
            if name != partition_name:
                in_names.append(name)
        elif alloc.kind == "ExternalOutput":
            shape = tuple(alloc.tensor_shape)
            dtype = mybir.dt.np(alloc.dtype)
            out_names.append(name)
            out_avals.append(jax.core.ShapedArray(shape, dtype))
            zero_outs.append(np.zeros(shape, dtype))
    n_params, n_outs = len(in_names), len(out_names)
    all_names = in_names + out_names + (
        [partition_name] if partition_name else [])

    def _body(*args):
        operands = list(args)
        if partition_name is not None:
            operands.append(b2j.partition_id_tensor())
        return tuple(b2j._bass_exec_p.bind(
            *operands, out_avals=tuple(out_avals), in_names=tuple(all_names),
            out_names=tuple(out_names), lowering_input_output_aliases=(),
            sim_require_finite=True, sim_require_nnan=True, nc=nc))

    devices = jax.devices()[:n_cores]
    mesh = Mesh(np.asarray(devices), ("core",))
    sharded = jax.jit(
        shard_map(_body, mesh=mesh,
                  in_specs=(PartitionSpec("core"),) * (n_params + n_outs),
                  out_specs=(PartitionSpec("core"),) * n_outs,
                  check_rep=False),
        donate_argnums=tuple(range(n_params, n_params + n_outs)),
        keep_unused=True)

    state = {"outs": [
        jax.device_put(np.zeros((n_cores * z.shape[0], *z.shape[1:]),
                                z.dtype),
                       jax.sharding.NamedSharding(mesh, PartitionSpec("core")))
        for z in zero_outs]}
    def run(ref_all, tgt_all):
        ins = [ref_all if nm == "ref" else tgt_all for nm in in_names]
        res = sharded(*ins, *state["outs"])
        out_np = np.asarray(res[0])
        state["outs"] = list(res)
        return out_np

    _RUN = run
    return _RUN


def make_in_maps(reference_fm: np.ndarray, target_fm: np.ndarray):
    """Per-core input dicts (for benchmarking harnesses)."""
    prep, _ = _get_host_fns()
    r, t, _, _ = prep(np.asarray(reference_fm, np.float32),
                      np.asarray(target_fm, np.float32))
    r = np.asarray(r).reshape(8, C, NBLK, 128)
    t = np.asarray(t).reshape(8, C, GY, GX)
    return [{"ref": r[c], "tgt": t[c]} for c in range(8)]


def assemble(results, reference_fm, target_fm) -> np.ndarray:
    """Host gather + norm/sqrt/mask for per-core result dicts (bench path)."""
    prep, finish = _get_host_fns()
    _, _, nr2, nt2p = prep(np.asarray(reference_fm, np.float32),
                           np.asarray(target_fm, np.float32))
    bands = np.stack([np.asarray(results[c]["out"]) for c in range(8)])
    out = finish(bands.reshape(8 * NSLAB, NPAIR, 16, NU, 4 * PBW),
                 nr2, nt2p)
    return np.asarray(out)


def kernel(reference_fm: np.ndarray, target_fm: np.ndarray) -> np.ndarray:
    prep, finish = _get_host_fns()
    r, t, nr2, nt2p = prep(np.asarray(reference_fm, np.float32),
                           np.asarray(target_fm, np.float32))
    run = _get_runner()
    bands = run(np.asarray(r), np.asarray(t))
    return np.asarray(finish(bands, nr2, nt2p))
